# revision 11
# baseline (speedup 1.0000x reference)
"""Trainium2 Bass kernel for nn_Attn: out = softmax_s(v . (W @ q_s + b)).

Algebraic identity:
    energies[s] = v . (W @ q[s] + b) = q[s] . (W^T v) + (v . b)
The (v . b) term is constant and softmax is shift-invariant, so it drops out.
u = W^T v is tiny (H=1024 values, 1/32 of the input bytes / FLOPs); it is
computed on the host in fp32 (alongside the existing host fp16 cast of q)
and shipped replicated across partitions, so the device critical path is
purely the q stream: energies = question @ u (a matvec) + sharded softmax.

Engine split for the 32 [128, 1024] tiles (tensor_tensor_reduce is fatal on
this HW — crashes the exec unit; STT/tensor_reduce are DVE-only per the
compiler's engine check, and gpsimd tensor_reduce is partition-axis only):
  D tiles (12): fused scalar_tensor_tensor on DVE            (~1.22 us)
  A tiles (12): TT mult on DVE (2x, ~0.69) + ACT Copy+accum  (~1.15)
  M tiles (8):  TT mult on GPSIMD (~2.0) + ACT Copy+accum    (~1.15)
balanced so DVE ~23 us, ACT ~23.5 us, GPSIMD ~16 us.

Chunks are graduated: 1-2 tile chunks first (fast compute ramp — the first
tile lands ~11 us instead of ~15), 4-tile chunks mid-stream, 1-2 tile
chunks last (short tail). Softmax is split in two groups (host merges
per-group stats): group A (tiles 0..29) reduces+exps while the last chunks
stream; only group B (tiles 30, 31) sits in the tail.

Precision: q streams as fp16 (host-cast), u is host-rounded to fp16; all
accumulations fp32. ~3e-5 scale-relative error, gate is 2e-2.

Why NO collectives: on this runner the 8 NEFFs enter ~60 us apart, so ANY
cross-core exchange stalls early cores by the skew. Cores are fully
independent; the host does the standard sharded-softmax merge.

Distribution over 8 NeuronCores — seq (token) sharding, q in its NATIVE
[tokens, H] layout: core r owns tokens [r*4096, (r+1)*4096); partition p
holds tokens [32p, 32p+32). Output [128, 36]: 32 unnormalized exp columns
+ (negmax_A, sum_A, negmax_B, sum_B); token of (r, p, t) = r*4096+32p+t.
"""

import numpy as np

S = 32768
H = 1024
NCORES = 8
TPC = S // NCORES  # 4096 tokens per core
TPT = 32  # tokens (sub-tiles) per partition
GB = 30  # group-B boundary: tiles [GB, TPT) get their own softmax stats

# chunk schedule: (queue, n_tiles); queues: 0=sync, 1=scalar, 2=pool
# (3rd HWDGE queue on the gpsimd engine — probes whether aggregate DMA
# exceeds the 2-queue ~425 GB/s)
CHUNKS = [
    (0, 1), (1, 2), (2, 1), (0, 2), (1, 3), (2, 2), (0, 4), (1, 4), (2, 2),
    (0, 4), (1, 3), (0, 2), (1, 1), (0, 1),
]
assert sum(n for _, n in CHUNKS) == TPT

# per-tile engine assignment in arrival order:
#   'D' fused STT on DVE, 'A' TT-mult on DVE + reduce on ACT, 'G' fused on
#   GPSIMD. a=6 D, b=17 A, g=9 G; tile 31 is D so the tail is one fused op,
#   tile 30 on GPSIMD so it overlaps the final D.
ASSIGN = ["A"] * TPT
for i in (1, 5, 9, 13, 17, 21, 25, 28):
    ASSIGN[i] = "M"
for i in (2, 4, 7, 10, 12, 15, 19, 22, 24, 27, 30, 31):
    ASSIGN[i] = "D"
assert len(ASSIGN) == TPT and ASSIGN.count("M") == 8 and ASSIGN.count("D") == 12

_cached = {}


def _build():
    from contextlib import ExitStack

    import concourse.bass as bass
    import concourse.mybir as mybir
    import concourse.tile as tile
    from concourse import bacc

    f32 = mybir.dt.float32
    f16 = mybir.dt.float16
    AX = mybir.AxisListType
    OP = mybir.AluOpType
    ds = bass.ds

    nc = bacc.Bacc(
        "TRN2", target_bir_lowering=False, debug=False, num_devices=NCORES
    )

    q = nc.dram_tensor("q", [TPC, H], f16, kind="ExternalInput")
    ur = nc.dram_tensor("ur", [128, H], f16, kind="ExternalInput")
    outp = nc.dram_tensor("outp", [128, TPT + 4], f32, kind="ExternalOutput")

    with tile.TileContext(nc) as tc, ExitStack() as ctx:
        const = ctx.enter_context(tc.tile_pool(name="const", bufs=1))
        qpool = ctx.enter_context(tc.tile_pool(name="qpool", bufs=1))
        work = ctx.enter_context(tc.tile_pool(name="work", bufs=1))
        scr = ctx.enter_context(tc.tile_pool(name="scr", bufs=2))

        # u_rep leads the pool queue (idle engine, lands ~9 us)
        ur_sb = const.tile([128, H], f16)
        nc.gpsimd.dma_start(ur_sb[:], ur[:])

        q_view = q[:].rearrange("(p t) h -> p (t h)", p=128)
        # graduated chunk DMAs
        chunk_tiles = []  # (sbuf tile, first tile idx, ntiles)
        t0 = 0
        for qi, ntile in CHUNKS:
            cw = ntile * H
            t_ = qpool.tile([128, cw], f16, tag=f"q{t0}", bufs=1)
            eng = [nc.sync, nc.scalar, nc.gpsimd][qi]
            eng.dma_start(t_[:], q_view[:, ds(t0 * H, cw)])
            chunk_tiles.append((t_, t0, ntile))
            t0 += ntile

        # --- energies
        e_loc = work.tile([128, TPT], f32)
        for t_, tbase, ntile in chunk_tiles:
            for s_ in range(ntile):
                t_idx = tbase + s_
                kind = ASSIGN[t_idx]
                src = t_[:, ds(s_ * H, H)]
                if kind == "A":
                    prod = scr.tile([128, H], f16, tag="proda", bufs=4)
                    nc.vector.tensor_tensor(prod[:], src, ur_sb[:], op=OP.mult)
                    junk = scr.tile([128, H], f16, tag="junk", bufs=2)
                    nc.scalar.activation(
                        junk[:], prod[:], mybir.ActivationFunctionType.Copy,
                        accum_out=e_loc[:, ds(t_idx, 1)],
                    )
                elif kind == "D":
                    prod = scr.tile([128, H], f16, tag="prodD", bufs=2)
                    nc.vector.scalar_tensor_tensor(
                        out=prod[:], in0=src, scalar=1.0, in1=ur_sb[:],
                        op0=OP.mult, op1=OP.mult,
                        accum_out=e_loc[:, ds(t_idx, 1)],
                    )
                else:  # M: mult on gpsimd (Pool), reduce on ACT
                    prod = scr.tile([128, H], f16, tag="prodM", bufs=2)
                    nc.gpsimd.tensor_tensor(prod[:], src, ur_sb[:], op=OP.mult)
                    junk = scr.tile([128, H], f16, tag="junkM", bufs=2)
                    nc.scalar.activation(
                        junk[:], prod[:], mybir.ActivationFunctionType.Copy,
                        accum_out=e_loc[:, ds(t_idx, 1)],
                    )

        # --- softmax pieces in two groups; host merges group stats.
        # ot: [exp(e - m_A) for tiles < GB | exp(e - m_B) for >= GB |
        #      -m_A, sum_A, -m_B, sum_B]
        ot = work.tile([128, TPT + 4], f32)
        nc.vector.tensor_reduce(
            ot[:, ds(TPT, 1)], e_loc[:, ds(0, GB)], axis=AX.X, op=OP.max,
            negate=True,
        )
        nc.scalar.activation(
            ot[:, ds(0, GB)], e_loc[:, ds(0, GB)],
            mybir.ActivationFunctionType.Exp,
            bias=ot[:, ds(TPT, 1)], scale=1.0,
            accum_out=ot[:, ds(TPT + 1, 1)],
        )
        nc.vector.tensor_reduce(
            ot[:, ds(TPT + 2, 1)], e_loc[:, ds(GB, TPT - GB)], axis=AX.X,
            op=OP.max, negate=True,
        )
        nc.scalar.activation(
            ot[:, ds(GB, TPT - GB)], e_loc[:, ds(GB, TPT - GB)],
            mybir.ActivationFunctionType.Exp,
            bias=ot[:, ds(TPT + 2, 1)], scale=1.0,
            accum_out=ot[:, ds(TPT + 3, 1)],
        )
        nc.sync.dma_start(outp[:], ot[:])

    nc.compile()
    return nc


def _get_nc():
    if "nc" not in _cached:
        _cached["nc"] = _build()
    return _cached["nc"]


def make_in_maps(question, W, v):
    qn = np.asarray(question)
    Wn = np.ascontiguousarray(np.asarray(W, dtype=np.float32))
    vn = np.ascontiguousarray(np.asarray(v, dtype=np.float32))
    q16 = np.ascontiguousarray(qn.astype(np.float16))
    # u = W^T v, host fp32 matvec, rounded to fp16 and replicated across
    # the 128 partitions so it lands DMA-ready
    u16 = (Wn.T @ vn).astype(np.float16)
    urm = np.ascontiguousarray(np.broadcast_to(u16[None, :], (128, H)))
    in_maps = []
    for r in range(NCORES):
        in_maps.append(
            {
                "q": q16[r * TPC : (r + 1) * TPC],  # contiguous row-slice view
                "ur": urm,
            }
        )
    return in_maps


def run(question, W, v, **spmd_kwargs):
    """Run the SPMD kernel; returns (out [S] fp32, BassKernelResults)."""
    from concourse.bass_utils import run_bass_kernel_spmd

    nc = _get_nc()
    in_maps = make_in_maps(question, W, v)
    res = run_bass_kernel_spmd(nc, in_maps, core_ids=list(range(NCORES)), **spmd_kwargs)
    blocks = np.stack(
        [
            np.asarray(res.results[r]["outp"], dtype=np.float64).reshape(
                128, TPT + 4
            )
            for r in range(NCORES)
        ]
    )  # [8, 128, 36]; token of (r, p, t) = r*4096 + 32p + t
    p_un = blocks[:, :, :TPT]
    m = np.stack([-blocks[:, :, TPT], -blocks[:, :, TPT + 2]], axis=-1)
    sums = np.stack([blocks[:, :, TPT + 1], blocks[:, :, TPT + 3]], axis=-1)
    M = m.max()
    wgt = np.exp(m - M)  # [8, 128, 2]
    Sg = (sums * wgt).sum()
    scale = np.concatenate(
        [
            np.repeat(wgt[:, :, :1], GB, axis=2),
            np.repeat(wgt[:, :, 1:], TPT - GB, axis=2),
        ],
        axis=2,
    )
    out = (p_un * scale / Sg).reshape(S)
    return out.astype(np.float32), res


def kernel(question, W, b, v):
    out, _ = run(question, W, v)
    return out.reshape(1, 1, S)


# revision 12
# speedup vs baseline: 1.0093x; 1.0093x over previous
"""Trainium2 Bass kernel for nn_Attn: out = softmax_s(v . (W @ q_s + b)).

Algebraic identity:
    energies[s] = v . (W @ q[s] + b) = q[s] . (W^T v) + (v . b)
The (v . b) term is constant and softmax is shift-invariant, so it drops out.
u = W^T v is tiny (H=1024 values, 1/32 of the input bytes / FLOPs); it is
computed on the host in fp32 (alongside the existing host fp16 cast of q)
and shipped replicated across partitions, so the device critical path is
purely the q stream: energies = question @ u (a matvec) + sharded softmax.

Engine split for the 32 [128, 1024] tiles (tensor_tensor_reduce is fatal on
this HW — crashes the exec unit; STT/tensor_reduce are DVE-only per the
compiler's engine check; gpsimd TT runs but slows concurrent DVE ops 2.6x
via SBUF port contention, a measured net loss):
  D tiles (9):  fused scalar_tensor_tensor on DVE            (~1.22 us)
  A tiles (23): TT mult on DVE (2x, ~0.69) + ACT Copy+accum  (~1.15)
balanced so DVE ~26.9 us, ACT ~26.4 us.

Chunks are graduated: 1-2 tile chunks first (fast compute ramp — the first
tile lands ~11 us instead of ~15), 4-tile chunks mid-stream, 1-2 tile
chunks last (short tail). Softmax is split in two groups (host merges
per-group stats): group A (tiles 0..29) reduces+exps while the last chunks
stream; only group B (tiles 30, 31) sits in the tail.

Precision: q streams as fp16 (host-cast), u is host-rounded to fp16; all
accumulations fp32. ~3e-5 scale-relative error, gate is 2e-2.

Why NO collectives: on this runner the 8 NEFFs enter ~60 us apart, so ANY
cross-core exchange stalls early cores by the skew. Cores are fully
independent; the host does the standard sharded-softmax merge.

Distribution over 8 NeuronCores — seq (token) sharding, q in its NATIVE
[tokens, H] layout: core r owns tokens [r*4096, (r+1)*4096); partition p
holds tokens [32p, 32p+32). Output [128, 36]: 32 unnormalized exp columns
+ (negmax_A, sum_A, negmax_B, sum_B); token of (r, p, t) = r*4096+32p+t.
"""

import numpy as np

S = 32768
H = 1024
NCORES = 8
TPC = S // NCORES  # 4096 tokens per core
TPT = 32  # tokens (sub-tiles) per partition
GB = 30  # group-B boundary: tiles [GB, TPT) get their own softmax stats

# chunk schedule: (queue, n_tiles); queues: 0=sync, 1=scalar. A 3rd
# (gpsimd) q-queue was measured to DROP aggregate DMA from ~425 to
# ~330 GB/s — two HWDGE queues is the sweet spot. The pool queue carries
# only the tiny ur transfer.
CHUNKS = [
    (0, 1), (1, 2), (0, 2), (1, 4), (0, 4), (1, 4), (0, 4), (1, 4), (0, 4),
    (1, 2), (0, 1),
]
assert sum(n for _, n in CHUNKS) == TPT

# per-tile engine assignment in arrival order:
#   'D' fused STT on DVE, 'A' TT-mult on DVE + reduce on ACT, 'G' fused on
#   GPSIMD. a=6 D, b=17 A, g=9 G; tile 31 is D so the tail is one fused op,
#   tile 30 on GPSIMD so it overlaps the final D.
ASSIGN = ["A"] * TPT
for i in (2, 6, 10, 13, 17, 20, 24, 28, 31):
    ASSIGN[i] = "D"
assert len(ASSIGN) == TPT and ASSIGN.count("D") == 9

_cached = {}


def _build():
    from contextlib import ExitStack

    import concourse.bass as bass
    import concourse.mybir as mybir
    import concourse.tile as tile
    from concourse import bacc

    f32 = mybir.dt.float32
    f16 = mybir.dt.float16
    AX = mybir.AxisListType
    OP = mybir.AluOpType
    ds = bass.ds

    nc = bacc.Bacc(
        "TRN2", target_bir_lowering=False, debug=False, num_devices=NCORES
    )

    q = nc.dram_tensor("q", [TPC, H], f16, kind="ExternalInput")
    ur = nc.dram_tensor("ur", [128, H], f16, kind="ExternalInput")
    outp = nc.dram_tensor("outp", [128, TPT + 4], f32, kind="ExternalOutput")

    with tile.TileContext(nc) as tc, ExitStack() as ctx:
        const = ctx.enter_context(tc.tile_pool(name="const", bufs=1))
        qpool = ctx.enter_context(tc.tile_pool(name="qpool", bufs=1))
        work = ctx.enter_context(tc.tile_pool(name="work", bufs=1))
        scr = ctx.enter_context(tc.tile_pool(name="scr", bufs=2))

        # u_rep leads the pool queue (idle engine, lands ~9 us)
        ur_sb = const.tile([128, H], f16)
        nc.gpsimd.dma_start(ur_sb[:], ur[:])

        q_view = q[:].rearrange("(p t) h -> p (t h)", p=128)
        # graduated chunk DMAs
        chunk_tiles = []  # (sbuf tile, first tile idx, ntiles)
        t0 = 0
        for qi, ntile in CHUNKS:
            cw = ntile * H
            t_ = qpool.tile([128, cw], f16, tag=f"q{t0}", bufs=1)
            eng = [nc.sync, nc.scalar, nc.gpsimd][qi]
            eng.dma_start(t_[:], q_view[:, ds(t0 * H, cw)])
            chunk_tiles.append((t_, t0, ntile))
            t0 += ntile

        # --- energies
        e_loc = work.tile([128, TPT], f32)
        for t_, tbase, ntile in chunk_tiles:
            for s_ in range(ntile):
                t_idx = tbase + s_
                kind = ASSIGN[t_idx]
                src = t_[:, ds(s_ * H, H)]
                if kind == "A":
                    prod = scr.tile([128, H], f16, tag="proda", bufs=4)
                    nc.vector.tensor_tensor(prod[:], src, ur_sb[:], op=OP.mult)
                    junk = scr.tile([128, H], f16, tag="junk", bufs=2)
                    nc.scalar.activation(
                        junk[:], prod[:], mybir.ActivationFunctionType.Copy,
                        accum_out=e_loc[:, ds(t_idx, 1)],
                    )
                else:  # D: fused mult+accum on DVE
                    prod = scr.tile([128, H], f16, tag="prodD", bufs=2)
                    nc.vector.scalar_tensor_tensor(
                        out=prod[:], in0=src, scalar=1.0, in1=ur_sb[:],
                        op0=OP.mult, op1=OP.mult,
                        accum_out=e_loc[:, ds(t_idx, 1)],
                    )

        # --- softmax pieces in two groups; host merges group stats.
        # ot: [exp(e - m_A) for tiles < GB | exp(e - m_B) for >= GB |
        #      -m_A, sum_A, -m_B, sum_B]
        ot = work.tile([128, TPT + 4], f32)
        nc.vector.tensor_reduce(
            ot[:, ds(TPT, 1)], e_loc[:, ds(0, GB)], axis=AX.X, op=OP.max,
            negate=True,
        )
        nc.scalar.activation(
            ot[:, ds(0, GB)], e_loc[:, ds(0, GB)],
            mybir.ActivationFunctionType.Exp,
            bias=ot[:, ds(TPT, 1)], scale=1.0,
            accum_out=ot[:, ds(TPT + 1, 1)],
        )
        nc.vector.tensor_reduce(
            ot[:, ds(TPT + 2, 1)], e_loc[:, ds(GB, TPT - GB)], axis=AX.X,
            op=OP.max, negate=True,
        )
        nc.scalar.activation(
            ot[:, ds(GB, TPT - GB)], e_loc[:, ds(GB, TPT - GB)],
            mybir.ActivationFunctionType.Exp,
            bias=ot[:, ds(TPT + 2, 1)], scale=1.0,
            accum_out=ot[:, ds(TPT + 3, 1)],
        )
        nc.sync.dma_start(outp[:], ot[:])

    nc.compile()
    return nc


def _get_nc():
    if "nc" not in _cached:
        _cached["nc"] = _build()
    return _cached["nc"]


def make_in_maps(question, W, v):
    qn = np.asarray(question)
    Wn = np.ascontiguousarray(np.asarray(W, dtype=np.float32))
    vn = np.ascontiguousarray(np.asarray(v, dtype=np.float32))
    q16 = np.ascontiguousarray(qn.astype(np.float16))
    # u = W^T v, host fp32 matvec, rounded to fp16 and replicated across
    # the 128 partitions so it lands DMA-ready
    u16 = (Wn.T @ vn).astype(np.float16)
    urm = np.ascontiguousarray(np.broadcast_to(u16[None, :], (128, H)))
    in_maps = []
    for r in range(NCORES):
        in_maps.append(
            {
                "q": q16[r * TPC : (r + 1) * TPC],  # contiguous row-slice view
                "ur": urm,
            }
        )
    return in_maps


def run(question, W, v, **spmd_kwargs):
    """Run the SPMD kernel; returns (out [S] fp32, BassKernelResults)."""
    from concourse.bass_utils import run_bass_kernel_spmd

    nc = _get_nc()
    in_maps = make_in_maps(question, W, v)
    res = run_bass_kernel_spmd(nc, in_maps, core_ids=list(range(NCORES)), **spmd_kwargs)
    blocks = np.stack(
        [
            np.asarray(res.results[r]["outp"], dtype=np.float64).reshape(
                128, TPT + 4
            )
            for r in range(NCORES)
        ]
    )  # [8, 128, 36]; token of (r, p, t) = r*4096 + 32p + t
    p_un = blocks[:, :, :TPT]
    m = np.stack([-blocks[:, :, TPT], -blocks[:, :, TPT + 2]], axis=-1)
    sums = np.stack([blocks[:, :, TPT + 1], blocks[:, :, TPT + 3]], axis=-1)
    M = m.max()
    wgt = np.exp(m - M)  # [8, 128, 2]
    Sg = (sums * wgt).sum()
    scale = np.concatenate(
        [
            np.repeat(wgt[:, :, :1], GB, axis=2),
            np.repeat(wgt[:, :, 1:], TPT - GB, axis=2),
        ],
        axis=2,
    )
    out = (p_un * scale / Sg).reshape(S)
    return out.astype(np.float32), res


def kernel(question, W, b, v):
    out, _ = run(question, W, v)
    return out.reshape(1, 1, S)


# revision 16
# speedup vs baseline: 1.2020x; 1.1909x over previous
"""Trainium2 Bass kernel for nn_Attn: out = softmax_s(v . (W @ q_s + b)).

Algebraic identity:
    energies[s] = v . (W @ q[s] + b) = q[s] . (W^T v) + (v . b)
The (v . b) term is constant and softmax is shift-invariant, so it drops out.
u = W^T v is tiny (H=1024 values, 1/32 of the input bytes / FLOPs); it is
computed on the host in fp32 (alongside the existing host fp16 cast of q)
and shipped replicated across partitions. The device computes the raw
energies (the 64 MB -> 128 KB matvec reduction, the entire data-parallel
workload); the softmax normalization — which is inherently GLOBAL across
all 8 independent cores — happens in the host merge step, in fp64.

Work split across THREE engine groups, all hidden under the ~20 us q DMA
stream (two HWDGE queues, ~212 GB/s each; a 3rd queue measurably HURTS):
  - 20 "normal" tiles [128 tok-grp, 1024 h] for DVE+ACT:
      D tiles (7):  fused scalar_tensor_tensor on DVE (~1.3 us incl
                    DVE_READ_ACCUMULATOR)
      A tiles (13): TT mult on DVE (2x_1p, ~0.69) + ACT Copy+accum
                    (~1.43 us incl ACTIVATION_READ_ACCUMULATOR)
  - 1536 tokens for the PE as HOST-TRANSPOSED qT blocks [128 h, 256 tok]
    (hc-major): lhsT = uT column [128, 1], 8 accumulating matmuls per
    block into PSUM [1, 256]; DVE/ACT copy each block's raw fp32 energies
    to SBUF mid-stream (DMA cannot read PSUM), one [1, 1536] DMA out.
(tensor_tensor_reduce crashes this HW; STT/tensor_reduce are DVE-only per
the compiler engine check; gpsimd TT slows concurrent DVE ops 2.6x via
SBUF port contention and gpsimd cannot touch PSUM — all measured/checked,
all rejected.)

Tail discipline: no device exp. The final chain is just
last-tile-STT -> [128, 20] output DMA, everything else lands earlier.

Precision: q streams as fp16 (host-cast), u host-rounded to fp16; energies
accumulate fp32; softmax in fp64 on host. ~3e-5 scale-rel, gate is 2e-2.

Why NO collectives: on this runner the 8 NEFFs enter ~60 us apart, so ANY
cross-core exchange stalls early cores by the skew. Cores are fully
independent.

Token layout, core r (tokens r*4096 ..): normal part = first 2560 tokens,
partition p holds tokens [20p, 20p+20); PE part = tokens 2560..4095.
outp [128, 20] raw energies; outp2 [1, 1536] raw PE energies.
"""

import numpy as np

S = 32768
H = 1024
NCORES = 8
TPC = S // NCORES  # 4096 tokens per core
NT = 20  # normal tiles (tokens per partition in the normal part)
NTOK = 128 * NT  # 2560 normal tokens per core
PE_TOK = TPC - NTOK  # 1536 PE tokens per core
PE_BLK = 256  # tokens per PE PSUM block
NBLK = PE_TOK // PE_BLK  # 6
OC = H // 128  # 8 h-chunks

# chunk schedule, arrival-interleaved; entries: (queue, kind, n)
#   queue: 0=sync 1=scalar; kind 'N': n normal tiles; 'T': one 256-token
#   qT block (2 tile-units). qT blocks sit mid-early so the PE (and the
#   PSUM->SBUF copies) finish before the tail.
CHUNKS = [
    (0, "N", 1), (1, "N", 2), (0, "T", 1), (1, "T", 1), (0, "T", 1),
    (1, "N", 2), (0, "N", 2), (1, "T", 1), (0, "N", 2), (1, "T", 1),
    (0, "T", 1), (1, "N", 2), (0, "N", 2), (1, "N", 2), (0, "N", 2),
    (0, "N", 1), (1, "N", 2),
]
assert sum(n for q, k, n in CHUNKS if k == "N") == NT
assert sum(1 for q, k, n in CHUNKS if k == "T") == NBLK

# normal-tile engine assignment (tile index = arrival order):
# 7 D (fused DVE), 13 A (DVE mult + ACT reduce); last tile D (fused tail).
ASSIGN = ["A"] * NT
for i in (2, 5, 8, 11, 14, 17, 19):
    ASSIGN[i] = "D"
assert ASSIGN.count("D") == 7

# PSUM->SBUF copies per PE block (0..5): early blocks on ACT (slack
# early), late blocks on DVE. Each copy is emitted into its engine's
# queue after the normal tile index below (so the in-order engine queue
# never stalls on a not-yet-finished PE block).
ACT_COPY_AFTER = {5: 0, 7: 1}
DVE_COPY_AFTER = {10: 2, 12: 3, 14: 4, 16: 5}

_cached = {}


def _build():
    from contextlib import ExitStack

    import concourse.bass as bass
    import concourse.mybir as mybir
    import concourse.tile as tile
    from concourse import bacc

    f32 = mybir.dt.float32
    f16 = mybir.dt.float16
    OP = mybir.AluOpType
    ds = bass.ds

    nc = bacc.Bacc(
        "TRN2", target_bir_lowering=False, debug=False, num_devices=NCORES
    )

    q = nc.dram_tensor("q", [NTOK, H], f16, kind="ExternalInput")
    qt = nc.dram_tensor("qt", [128, OC * PE_TOK], f16, kind="ExternalInput")
    # ur = [u replicated [128, H] | uT [128, OC]]
    ur = nc.dram_tensor("ur", [128, H + OC], f16, kind="ExternalInput")
    outp = nc.dram_tensor("outp", [128, NT], f32, kind="ExternalOutput")
    outp2 = nc.dram_tensor("outp2", [1, PE_TOK], f32, kind="ExternalOutput")

    with tile.TileContext(nc) as tc, ExitStack() as ctx:
        const = ctx.enter_context(tc.tile_pool(name="const", bufs=1))
        qpool = ctx.enter_context(tc.tile_pool(name="qpool", bufs=1))
        work = ctx.enter_context(tc.tile_pool(name="work", bufs=1))
        scr = ctx.enter_context(tc.tile_pool(name="scr", bufs=2))
        psum = ctx.enter_context(tc.tile_pool(name="psum", bufs=6, space="PSUM"))

        # ur leads the sync queue (tiny: 258 KB)
        ur_sb = const.tile([128, H + OC], f16)
        nc.sync.dma_start(ur_sb[:], ur[:])

        q_view = q[:].rearrange("(p t) h -> p (t h)", p=128)
        n_seen = 0
        t_seen = 0
        norm_chunks = []  # (sbuf tile, first tile idx, ntiles)
        qt_chunks = []  # (sbuf tile, block idx)
        for qi, kind, n in CHUNKS:
            eng = nc.sync if qi == 0 else nc.scalar
            if kind == "N":
                cw = n * H
                t_ = qpool.tile([128, cw], f16, tag=f"qn{n_seen}", bufs=1)
                eng.dma_start(t_[:], q_view[:, ds(n_seen * H, cw)])
                norm_chunks.append((t_, n_seen, n))
                n_seen += n
            else:
                cw = OC * PE_BLK
                t_ = qpool.tile([128, cw], f16, tag=f"qt{t_seen}", bufs=1)
                eng.dma_start(t_[:], qt[:, ds(t_seen * cw, cw)])
                qt_chunks.append((t_, t_seen))
                t_seen += 1

        # --- PE path: per 256-token block, 8 accumulating rank-1 matmuls
        pe_ps = []
        for t_, blk in qt_chunks:
            pb = psum.tile([1, PE_BLK], f32, tag=f"pe{blk}", bufs=1)
            for hc in range(OC):
                nc.tensor.matmul(
                    pb[:],
                    lhsT=ur_sb[:, ds(H + hc, 1)],
                    rhs=t_[:, ds(hc * PE_BLK, PE_BLK)],
                    start=(hc == 0),
                    stop=(hc == OC - 1),
                )
            pe_ps.append(pb)
        pe_sb = work.tile([1, PE_TOK], f32)

        # --- normal tiles: raw energies into e_loc columns
        e_loc = work.tile([128, NT], f32)
        for t_, tbase, ntile in norm_chunks:
            for s_ in range(ntile):
                t_idx = tbase + s_
                src = t_[:, ds(s_ * H, H)]
                if ASSIGN[t_idx] == "A":
                    prod = scr.tile([128, H], f16, tag="proda", bufs=8)
                    nc.vector.tensor_tensor(prod[:], src, ur_sb[:, ds(0, H)],
                                            op=OP.mult)
                    junk = scr.tile([128, H], f16, tag="junk", bufs=2)
                    nc.scalar.activation(
                        junk[:], prod[:], mybir.ActivationFunctionType.Copy,
                        accum_out=e_loc[:, ds(t_idx, 1)],
                    )
                else:
                    prod = scr.tile([128, H], f16, tag="prodd", bufs=4)
                    nc.vector.scalar_tensor_tensor(
                        out=prod[:], in0=src, scalar=1.0,
                        in1=ur_sb[:, ds(0, H)],
                        op0=OP.mult, op1=OP.mult,
                        accum_out=e_loc[:, ds(t_idx, 1)],
                    )
                if t_idx in ACT_COPY_AFTER:
                    blk = ACT_COPY_AFTER[t_idx]
                    nc.scalar.activation(
                        pe_sb[:, ds(blk * PE_BLK, PE_BLK)], pe_ps[blk][:],
                        mybir.ActivationFunctionType.Copy,
                    )
                if t_idx in DVE_COPY_AFTER:
                    blk = DVE_COPY_AFTER[t_idx]
                    nc.vector.tensor_scalar_add(
                        pe_sb[:, ds(blk * PE_BLK, PE_BLK)], pe_ps[blk][:], 0.0
                    )

        nc.sync.dma_start(outp2[:], pe_sb[:])
        nc.sync.dma_start(outp[:], e_loc[:])

    nc.compile()
    return nc


def _get_nc():
    if "nc" not in _cached:
        _cached["nc"] = _build()
    return _cached["nc"]


def make_in_maps(question, W, v):
    qn = np.asarray(question)
    Wn = np.ascontiguousarray(np.asarray(W, dtype=np.float32))
    vn = np.ascontiguousarray(np.asarray(v, dtype=np.float32))
    q16 = np.ascontiguousarray(qn.astype(np.float16))
    u16 = (Wn.T @ vn).astype(np.float16)
    urm = np.empty((128, H + OC), dtype=np.float16)
    urm[:, :H] = u16[None, :]
    urm[:, H:] = u16.reshape(OC, 128).T  # uT[hp, hc] = u[hc*128 + hp]
    in_maps = []
    for r in range(NCORES):
        q_r = q16[r * TPC : (r + 1) * TPC]
        # PE part: [128 hp, block-major, hc-major within block]
        qt_r = np.ascontiguousarray(
            q_r[NTOK:]
            .reshape(NBLK, PE_BLK, OC, 128)
            .transpose(3, 0, 2, 1)
            .reshape(128, OC * PE_TOK)
        )
        in_maps.append({"q": np.ascontiguousarray(q_r[:NTOK]), "qt": qt_r,
                        "ur": urm})
    return in_maps


def run(question, W, v, **spmd_kwargs):
    """Run the SPMD kernel; returns (out [S] fp32, BassKernelResults)."""
    from concourse.bass_utils import run_bass_kernel_spmd

    nc = _get_nc()
    in_maps = make_in_maps(question, W, v)
    res = run_bass_kernel_spmd(nc, in_maps, core_ids=list(range(NCORES)), **spmd_kwargs)
    e = np.empty((NCORES, TPC), dtype=np.float64)
    for r in range(NCORES):
        e[r, :NTOK] = (
            np.asarray(res.results[r]["outp"], dtype=np.float64).reshape(NTOK)
        )
        e[r, NTOK:] = np.asarray(
            res.results[r]["outp2"], dtype=np.float64
        ).reshape(PE_TOK)
    ex = np.exp(e - e.max())
    out = (ex / ex.sum()).reshape(S)
    return out.astype(np.float32), res


def kernel(question, W, b, v):
    out, _ = run(question, W, v)
    return out.reshape(1, 1, S)


# revision 17
# speedup vs baseline: 1.3348x; 1.1105x over previous
"""Trainium2 Bass kernel for nn_Attn: out = softmax_s(v . (W @ q_s + b)).

Algebraic identity:
    energies[s] = v . (W @ q[s] + b) = q[s] . (W^T v) + (v . b)
The (v . b) term is constant and softmax is shift-invariant, so it drops out.
u = W^T v is tiny (H=1024 values, 1/32 of the input bytes / FLOPs); it is
computed on the host in fp32 (alongside the existing host fp16 cast of q)
and shipped replicated across partitions. The device computes the raw
energies (the 64 MB -> 128 KB matvec reduction, the entire data-parallel
workload); the softmax normalization — which is inherently GLOBAL across
all 8 independent cores — happens in the host merge step, in fp64.

Work split across THREE engine groups, all hidden under the ~20 us q DMA
stream (two HWDGE queues, ~212 GB/s each; a 3rd queue measurably HURTS):
  - 20 "normal" tiles [128 tok-grp, 1024 h] for DVE+ACT:
      D tiles (7):  fused scalar_tensor_tensor on DVE (~1.3 us incl
                    DVE_READ_ACCUMULATOR)
      A tiles (13): TT mult on DVE (2x_1p, ~0.69) + ACT Copy+accum
                    (~1.43 us incl ACTIVATION_READ_ACCUMULATOR)
  - 1536 tokens for the PE as HOST-TRANSPOSED qT blocks [128 h, 256 tok]
    (hc-major): lhsT = uT column [128, 1], 8 accumulating matmuls per
    block into PSUM [1, 256]; DVE/ACT copy each block's raw fp32 energies
    to SBUF mid-stream (DMA cannot read PSUM), one [1, 1536] DMA out.
(tensor_tensor_reduce crashes this HW; STT/tensor_reduce are DVE-only per
the compiler engine check; gpsimd TT slows concurrent DVE ops 2.6x via
SBUF port contention and gpsimd cannot touch PSUM — all measured/checked,
all rejected.)

Tail discipline: no device exp. The final chain is just
last-tile-STT -> [128, 20] output DMA, everything else lands earlier.

Precision: q streams as fp16 (host-cast), u host-rounded to fp16; energies
accumulate fp32; softmax in fp64 on host. ~3e-5 scale-rel, gate is 2e-2.

Why NO collectives: on this runner the 8 NEFFs enter ~60 us apart, so ANY
cross-core exchange stalls early cores by the skew. Cores are fully
independent.

Token layout, core r (tokens r*4096 ..): normal part = first 2560 tokens,
partition p holds tokens [20p, 20p+20); PE part = tokens 2560..4095.
outp [128, 20] raw energies; outp2 [1, 1536] raw PE energies.
"""

import numpy as np

S = 32768
H = 1024
NCORES = 8
TPC = S // NCORES  # 4096 tokens per core
NT = 18  # normal tiles (tokens per partition in the normal part)
NTOK = 128 * NT  # 2304 normal tokens per core
PE_TOK = TPC - NTOK  # 1792 PE tokens per core
PE_BLK = 256  # tokens per PE PSUM block
NBLK = PE_TOK // PE_BLK  # 7
OC = H // 128  # 8 h-chunks

# chunk schedule, arrival-interleaved; entries: (queue, kind, n)
#   queue: 0=sync 1=scalar; kind 'N': n normal tiles; 'T': one 256-token
#   qT block (2 tile-units). qT blocks sit mid-early so the PE (and the
#   PSUM->SBUF copies) finish before the tail.
# entries: (queue, kind, n): 'N' = n normal tiles, 'T' = n 256-token PE
# blocks. queue 0 = sync engine's HWDGE queue, 1 = gpsimd's (the scalar
# engine issues NO DMAs — its trigger time goes to reduces instead).
CHUNKS = [
    (0, "N", 1), (1, "T", 1), (0, "T", 1), (1, "N", 2), (0, "N", 2),
    (1, "N", 4), (0, "T", 1), (1, "T", 2), (0, "N", 2), (1, "N", 2),
    (0, "T", 1), (1, "N", 2), (0, "N", 2), (0, "T", 1), (0, "N", 1),
]
assert sum(n for q, k, n in CHUNKS if k == "N") == NT
assert sum(n for q, k, n in CHUNKS if k == "T") == NBLK

# normal-tile engine assignment (tile index = arrival order):
# 7 D (fused DVE), 13 A (DVE mult + ACT reduce); last tile D (fused tail).
ASSIGN = ["A"] * NT
for i in (2, 5, 8, 11, 14, 16, 17):
    ASSIGN[i] = "D"
assert ASSIGN.count("D") == 7

# PSUM->SBUF copies per PE block (0..5): early blocks on ACT (slack
# early), late blocks on DVE. Each copy is emitted into its engine's
# queue after the normal tile index below (so the in-order engine queue
# never stalls on a not-yet-finished PE block).
ACT_COPY_AFTER = {6: 0, 8: 1, 10: 2}
DVE_COPY_AFTER = {11: 3, 13: 4, 15: 5, 17: 6}

_cached = {}


def _build():
    from contextlib import ExitStack

    import concourse.bass as bass
    import concourse.mybir as mybir
    import concourse.tile as tile
    from concourse import bacc

    f32 = mybir.dt.float32
    f16 = mybir.dt.float16
    OP = mybir.AluOpType
    ds = bass.ds

    nc = bacc.Bacc(
        "TRN2", target_bir_lowering=False, debug=False, num_devices=NCORES
    )

    q = nc.dram_tensor("q", [NTOK, H], f16, kind="ExternalInput")
    qt = nc.dram_tensor("qt", [128, OC * PE_TOK], f16, kind="ExternalInput")
    # ur = [u replicated [128, H] | uT [128, OC]]
    ur = nc.dram_tensor("ur", [128, H + OC], f16, kind="ExternalInput")
    outp = nc.dram_tensor("outp", [128, NT], f32, kind="ExternalOutput")
    outp2 = nc.dram_tensor("outp2", [1, PE_TOK], f32, kind="ExternalOutput")

    with tile.TileContext(nc) as tc, ExitStack() as ctx:
        const = ctx.enter_context(tc.tile_pool(name="const", bufs=1))
        qpool = ctx.enter_context(tc.tile_pool(name="qpool", bufs=1))
        work = ctx.enter_context(tc.tile_pool(name="work", bufs=1))
        scr = ctx.enter_context(tc.tile_pool(name="scr", bufs=2))
        psum = ctx.enter_context(tc.tile_pool(name="psum", bufs=1, space="PSUM"))

        # ur leads the sync queue (tiny: 258 KB)
        ur_sb = const.tile([128, H + OC], f16)
        nc.sync.dma_start(ur_sb[:], ur[:])

        q_view = q[:].rearrange("(p t) h -> p (t h)", p=128)
        n_seen = 0
        t_seen = 0
        norm_chunks = []  # (sbuf tile, first tile idx, ntiles)
        qt_chunks = []  # (sbuf tile, block idx)
        for qi, kind, n in CHUNKS:
            eng = nc.sync if qi == 0 else nc.gpsimd
            if kind == "N":
                cw = n * H
                t_ = qpool.tile([128, cw], f16, tag=f"qn{n_seen}", bufs=1)
                eng.dma_start(t_[:], q_view[:, ds(n_seen * H, cw)])
                norm_chunks.append((t_, n_seen, n))
                n_seen += n
            else:
                cw = n * OC * PE_BLK
                t_ = qpool.tile([128, cw], f16, tag=f"qt{t_seen}", bufs=1)
                eng.dma_start(t_[:], qt[:, ds(t_seen * OC * PE_BLK, cw)])
                for j in range(n):
                    qt_chunks.append((t_, j, t_seen + j))
                t_seen += n

        # --- PE path: per 256-token block, 8 accumulating rank-1 matmuls
        pe_ps = [None] * NBLK
        for t_, j, blk in qt_chunks:
            pb = psum.tile([1, PE_BLK], f32, tag=f"pe{blk}", bufs=1)
            for hc in range(OC):
                nc.tensor.matmul(
                    pb[:],
                    lhsT=ur_sb[:, ds(H + hc, 1)],
                    rhs=t_[:, ds((j * OC + hc) * PE_BLK, PE_BLK)],
                    start=(hc == 0),
                    stop=(hc == OC - 1),
                )
            pe_ps[blk] = pb
        pe_sb = work.tile([1, PE_TOK], f32)

        # --- normal tiles: raw energies into e_loc columns
        e_loc = work.tile([128, NT], f32)
        for t_, tbase, ntile in norm_chunks:
            for s_ in range(ntile):
                t_idx = tbase + s_
                src = t_[:, ds(s_ * H, H)]
                if ASSIGN[t_idx] == "A":
                    prod = scr.tile([128, H], f16, tag="proda", bufs=8)
                    nc.vector.tensor_tensor(prod[:], src, ur_sb[:, ds(0, H)],
                                            op=OP.mult)
                    junk = scr.tile([128, H], f16, tag="junk", bufs=2)
                    nc.scalar.activation(
                        junk[:], prod[:], mybir.ActivationFunctionType.Copy,
                        accum_out=e_loc[:, ds(t_idx, 1)],
                    )
                else:
                    prod = scr.tile([128, H], f16, tag="prodd", bufs=4)
                    nc.vector.scalar_tensor_tensor(
                        out=prod[:], in0=src, scalar=1.0,
                        in1=ur_sb[:, ds(0, H)],
                        op0=OP.mult, op1=OP.mult,
                        accum_out=e_loc[:, ds(t_idx, 1)],
                    )
                if t_idx in ACT_COPY_AFTER:
                    blk = ACT_COPY_AFTER[t_idx]
                    nc.scalar.activation(
                        pe_sb[:, ds(blk * PE_BLK, PE_BLK)], pe_ps[blk][:],
                        mybir.ActivationFunctionType.Copy,
                    )
                if t_idx in DVE_COPY_AFTER:
                    blk = DVE_COPY_AFTER[t_idx]
                    nc.vector.tensor_scalar_add(
                        pe_sb[:, ds(blk * PE_BLK, PE_BLK)], pe_ps[blk][:], 0.0
                    )

        nc.sync.dma_start(outp2[:], pe_sb[:])
        nc.sync.dma_start(outp[:], e_loc[:])

    nc.compile()
    return nc


def _get_nc():
    if "nc" not in _cached:
        _cached["nc"] = _build()
    return _cached["nc"]


def make_in_maps(question, W, v):
    qn = np.asarray(question)
    Wn = np.ascontiguousarray(np.asarray(W, dtype=np.float32))
    vn = np.ascontiguousarray(np.asarray(v, dtype=np.float32))
    q16 = np.ascontiguousarray(qn.astype(np.float16))
    u16 = (Wn.T @ vn).astype(np.float16)
    urm = np.empty((128, H + OC), dtype=np.float16)
    urm[:, :H] = u16[None, :]
    urm[:, H:] = u16.reshape(OC, 128).T  # uT[hp, hc] = u[hc*128 + hp]
    in_maps = []
    for r in range(NCORES):
        q_r = q16[r * TPC : (r + 1) * TPC]
        # PE part: [128 hp, block-major, hc-major within block]
        qt_r = np.ascontiguousarray(
            q_r[NTOK:]
            .reshape(NBLK, PE_BLK, OC, 128)
            .transpose(3, 0, 2, 1)
            .reshape(128, OC * PE_TOK)
        )
        in_maps.append({"q": np.ascontiguousarray(q_r[:NTOK]), "qt": qt_r,
                        "ur": urm})
    return in_maps


def run(question, W, v, **spmd_kwargs):
    """Run the SPMD kernel; returns (out [S] fp32, BassKernelResults)."""
    from concourse.bass_utils import run_bass_kernel_spmd

    nc = _get_nc()
    in_maps = make_in_maps(question, W, v)
    res = run_bass_kernel_spmd(nc, in_maps, core_ids=list(range(NCORES)), **spmd_kwargs)
    e = np.empty((NCORES, TPC), dtype=np.float64)
    for r in range(NCORES):
        e[r, :NTOK] = (
            np.asarray(res.results[r]["outp"], dtype=np.float64).reshape(NTOK)
        )
        e[r, NTOK:] = np.asarray(
            res.results[r]["outp2"], dtype=np.float64
        ).reshape(PE_TOK)
    ex = np.exp(e - e.max())
    out = (ex / ex.sum()).reshape(S)
    return out.astype(np.float32), res


def kernel(question, W, b, v):
    out, _ = run(question, W, v)
    return out.reshape(1, 1, S)


# revision 18
# speedup vs baseline: 1.3645x; 1.0222x over previous
"""Trainium2 Bass kernel for nn_Attn: out = softmax_s(v . (W @ q_s + b)).

Algebraic identity:
    energies[s] = v . (W @ q[s] + b) = q[s] . (W^T v) + (v . b)
The (v . b) term is constant and softmax is shift-invariant, so it drops out.
u = W^T v is tiny (H=1024 values, 1/32 of the input bytes / FLOPs); it is
computed on the host in fp32 (alongside the existing host fp16 cast of q)
and shipped replicated across partitions. The device computes the raw
energies (the 64 MB -> 128 KB matvec reduction, the entire data-parallel
workload); the softmax normalization — which is inherently GLOBAL across
all 8 independent cores — happens in the host merge step, in fp64.

Work split across THREE engine groups, all hidden under the ~20 us q DMA
stream (two HWDGE queues, ~212 GB/s each; a 3rd queue measurably HURTS):
  - 20 "normal" tiles [128 tok-grp, 1024 h] for DVE+ACT:
      D tiles (7):  fused scalar_tensor_tensor on DVE (~1.3 us incl
                    DVE_READ_ACCUMULATOR)
      A tiles (13): TT mult on DVE (2x_1p, ~0.69) + ACT Copy+accum
                    (~1.43 us incl ACTIVATION_READ_ACCUMULATOR)
  - 1536 tokens for the PE as HOST-TRANSPOSED qT blocks [128 h, 256 tok]
    (hc-major): lhsT = uT column [128, 1], 8 accumulating matmuls per
    block into PSUM [1, 256]; DVE/ACT copy each block's raw fp32 energies
    to SBUF mid-stream (DMA cannot read PSUM), one [1, 1536] DMA out.
(tensor_tensor_reduce crashes this HW; STT/tensor_reduce are DVE-only per
the compiler engine check; gpsimd TT slows concurrent DVE ops 2.6x via
SBUF port contention and gpsimd cannot touch PSUM — all measured/checked,
all rejected.)

Tail discipline: no device exp. The final chain is just
last-tile-STT -> [128, 20] output DMA, everything else lands earlier.

Precision: q streams as fp16 (host-cast), u host-rounded to fp16; energies
accumulate fp32; softmax in fp64 on host. ~3e-5 scale-rel, gate is 2e-2.

Why NO collectives: on this runner the 8 NEFFs enter ~60 us apart, so ANY
cross-core exchange stalls early cores by the skew. Cores are fully
independent.

Token layout, core r (tokens r*4096 ..): normal part = first 2560 tokens,
partition p holds tokens [20p, 20p+20); PE part = tokens 2560..4095.
outp [128, 20] raw energies; outp2 [1, 1536] raw PE energies.
"""

import numpy as np

S = 32768
H = 1024
NCORES = 8
TPC = S // NCORES  # 4096 tokens per core
NT = 18  # normal tiles (tokens per partition in the normal part)
NTOK = 128 * NT  # 2304 normal tokens per core
PE_TOK = TPC - NTOK  # 1792 PE tokens per core
PE_BLK = 256  # tokens per PE PSUM block
NBLK = PE_TOK // PE_BLK  # 7
OC = H // 128  # 8 h-chunks

# chunk schedule, arrival-interleaved; entries: (queue, kind, n)
#   queue: 0=sync 1=scalar; kind 'N': n normal tiles; 'T': one 256-token
#   qT block (2 tile-units). qT blocks sit mid-early so the PE (and the
#   PSUM->SBUF copies) finish before the tail.
# entries: (queue, kind, n): 'N' = n normal tiles, 'T' = n 256-token PE
# blocks. Queues: 0 = sync engine (fine-grained, fast-start), 1 = scalar
# (only 3 BIG chunks — ~2 us of ACT trigger time), 2 = gpsimd (~110 GB/s,
# lightly loaded). 3 queues together saturate the ~437 GB/s HBM cap.
CHUNKS = [
    (0, "N", 1), (0, "T", 1), (2, "T", 1), (0, "T", 1), (1, "T", 2),
    (2, "N", 2), (1, "N", 4), (0, "N", 2), (0, "T", 1), (0, "N", 2),
    (2, "T", 1), (1, "N", 3), (0, "N", 2), (0, "N", 2),
]
assert sum(n for q, k, n in CHUNKS if k == "N") == NT
assert sum(n for q, k, n in CHUNKS if k == "T") == NBLK

# normal-tile engine assignment (tile index = arrival order):
# 7 D (fused DVE), 13 A (DVE mult + ACT reduce); last tile D (fused tail).
ASSIGN = ["A"] * NT
for i in (2, 5, 8, 11, 13, 15, 17):
    ASSIGN[i] = "D"
assert ASSIGN.count("D") == 7

# PSUM->SBUF copies per PE block (0..5): early blocks on ACT (slack
# early), late blocks on DVE. Each copy is emitted into its engine's
# queue after the normal tile index below (so the in-order engine queue
# never stalls on a not-yet-finished PE block).
ACT_COPY_AFTER = {6: 0, 8: 1, 10: 2}
DVE_COPY_AFTER = {11: 3, 13: 4, 15: 5, 17: 6}

_cached = {}


def _build():
    from contextlib import ExitStack

    import concourse.bass as bass
    import concourse.mybir as mybir
    import concourse.tile as tile
    from concourse import bacc

    f32 = mybir.dt.float32
    f16 = mybir.dt.float16
    OP = mybir.AluOpType
    ds = bass.ds

    nc = bacc.Bacc(
        "TRN2", target_bir_lowering=False, debug=False, num_devices=NCORES
    )

    q = nc.dram_tensor("q", [NTOK, H], f16, kind="ExternalInput")
    qt = nc.dram_tensor("qt", [128, OC * PE_TOK], f16, kind="ExternalInput")
    # ur = [u replicated [128, H] | uT [128, OC]]
    ur = nc.dram_tensor("ur", [128, H + OC], f16, kind="ExternalInput")
    outp = nc.dram_tensor("outp", [128, NT], f32, kind="ExternalOutput")
    outp2 = nc.dram_tensor("outp2", [1, PE_TOK], f32, kind="ExternalOutput")

    with tile.TileContext(nc) as tc, ExitStack() as ctx:
        const = ctx.enter_context(tc.tile_pool(name="const", bufs=1))
        qpool = ctx.enter_context(tc.tile_pool(name="qpool", bufs=1))
        work = ctx.enter_context(tc.tile_pool(name="work", bufs=1))
        scr = ctx.enter_context(tc.tile_pool(name="scr", bufs=2))
        psum = ctx.enter_context(tc.tile_pool(name="psum", bufs=1, space="PSUM"))

        # ur leads the sync queue (tiny: 258 KB)
        ur_sb = const.tile([128, H + OC], f16)
        nc.sync.dma_start(ur_sb[:], ur[:])

        q_view = q[:].rearrange("(p t) h -> p (t h)", p=128)
        n_seen = 0
        t_seen = 0
        norm_chunks = []  # (sbuf tile, first tile idx, ntiles)
        qt_chunks = []  # (sbuf tile, block idx)
        for qi, kind, n in CHUNKS:
            eng = [nc.sync, nc.scalar, nc.gpsimd][qi]
            if kind == "N":
                cw = n * H
                t_ = qpool.tile([128, cw], f16, tag=f"qn{n_seen}", bufs=1)
                eng.dma_start(t_[:], q_view[:, ds(n_seen * H, cw)])
                norm_chunks.append((t_, n_seen, n))
                n_seen += n
            else:
                cw = n * OC * PE_BLK
                t_ = qpool.tile([128, cw], f16, tag=f"qt{t_seen}", bufs=1)
                eng.dma_start(t_[:], qt[:, ds(t_seen * OC * PE_BLK, cw)])
                for j in range(n):
                    qt_chunks.append((t_, j, t_seen + j))
                t_seen += n

        # --- PE path: per 256-token block, 8 accumulating rank-1 matmuls
        pe_ps = [None] * NBLK
        for t_, j, blk in qt_chunks:
            pb = psum.tile([1, PE_BLK], f32, tag=f"pe{blk}", bufs=1)
            for hc in range(OC):
                nc.tensor.matmul(
                    pb[:],
                    lhsT=ur_sb[:, ds(H + hc, 1)],
                    rhs=t_[:, ds((j * OC + hc) * PE_BLK, PE_BLK)],
                    start=(hc == 0),
                    stop=(hc == OC - 1),
                )
            pe_ps[blk] = pb
        pe_sb = work.tile([1, PE_TOK], f32)

        # --- normal tiles: raw energies into e_loc columns
        e_loc = work.tile([128, NT], f32)
        for t_, tbase, ntile in norm_chunks:
            for s_ in range(ntile):
                t_idx = tbase + s_
                src = t_[:, ds(s_ * H, H)]
                if ASSIGN[t_idx] == "A":
                    prod = scr.tile([128, H], f16, tag="proda", bufs=8)
                    nc.vector.tensor_tensor(prod[:], src, ur_sb[:, ds(0, H)],
                                            op=OP.mult)
                    junk = scr.tile([128, H], f16, tag="junk", bufs=2)
                    nc.scalar.activation(
                        junk[:], prod[:], mybir.ActivationFunctionType.Copy,
                        accum_out=e_loc[:, ds(t_idx, 1)],
                    )
                else:
                    prod = scr.tile([128, H], f16, tag="prodd", bufs=4)
                    nc.vector.scalar_tensor_tensor(
                        out=prod[:], in0=src, scalar=1.0,
                        in1=ur_sb[:, ds(0, H)],
                        op0=OP.mult, op1=OP.mult,
                        accum_out=e_loc[:, ds(t_idx, 1)],
                    )
                if t_idx in ACT_COPY_AFTER:
                    blk = ACT_COPY_AFTER[t_idx]
                    nc.scalar.activation(
                        pe_sb[:, ds(blk * PE_BLK, PE_BLK)], pe_ps[blk][:],
                        mybir.ActivationFunctionType.Copy,
                    )
                if t_idx in DVE_COPY_AFTER:
                    blk = DVE_COPY_AFTER[t_idx]
                    nc.vector.tensor_scalar_add(
                        pe_sb[:, ds(blk * PE_BLK, PE_BLK)], pe_ps[blk][:], 0.0
                    )

        nc.sync.dma_start(outp[:], e_loc[:])
        nc.sync.dma_start(outp2[:], pe_sb[:])

    nc.compile()
    return nc


def _get_nc():
    if "nc" not in _cached:
        _cached["nc"] = _build()
    return _cached["nc"]


def make_in_maps(question, W, v):
    qn = np.asarray(question)
    Wn = np.ascontiguousarray(np.asarray(W, dtype=np.float32))
    vn = np.ascontiguousarray(np.asarray(v, dtype=np.float32))
    q16 = np.ascontiguousarray(qn.astype(np.float16))
    u16 = (Wn.T @ vn).astype(np.float16)
    urm = np.empty((128, H + OC), dtype=np.float16)
    urm[:, :H] = u16[None, :]
    urm[:, H:] = u16.reshape(OC, 128).T  # uT[hp, hc] = u[hc*128 + hp]
    in_maps = []
    for r in range(NCORES):
        q_r = q16[r * TPC : (r + 1) * TPC]
        # PE part: [128 hp, block-major, hc-major within block]
        qt_r = np.ascontiguousarray(
            q_r[NTOK:]
            .reshape(NBLK, PE_BLK, OC, 128)
            .transpose(3, 0, 2, 1)
            .reshape(128, OC * PE_TOK)
        )
        in_maps.append({"q": np.ascontiguousarray(q_r[:NTOK]), "qt": qt_r,
                        "ur": urm})
    return in_maps


def run(question, W, v, **spmd_kwargs):
    """Run the SPMD kernel; returns (out [S] fp32, BassKernelResults)."""
    from concourse.bass_utils import run_bass_kernel_spmd

    nc = _get_nc()
    in_maps = make_in_maps(question, W, v)
    res = run_bass_kernel_spmd(nc, in_maps, core_ids=list(range(NCORES)), **spmd_kwargs)
    e = np.empty((NCORES, TPC), dtype=np.float64)
    for r in range(NCORES):
        e[r, :NTOK] = (
            np.asarray(res.results[r]["outp"], dtype=np.float64).reshape(NTOK)
        )
        e[r, NTOK:] = np.asarray(
            res.results[r]["outp2"], dtype=np.float64
        ).reshape(PE_TOK)
    ex = np.exp(e - e.max())
    out = (ex / ex.sum()).reshape(S)
    return out.astype(np.float32), res


def kernel(question, W, b, v):
    out, _ = run(question, W, v)
    return out.reshape(1, 1, S)


# revision 19
# speedup vs baseline: 1.4091x; 1.0327x over previous
"""Trainium2 Bass kernel for nn_Attn: out = softmax_s(v . (W @ q_s + b)).

Algebraic identity:
    energies[s] = v . (W @ q[s] + b) = q[s] . (W^T v) + (v . b)
The (v . b) term is constant and softmax is shift-invariant, so it drops out.
u = W^T v is tiny (H=1024 values, 1/32 of the input bytes / FLOPs); it is
computed on the host in fp32 (alongside the existing host fp16 cast of q)
and shipped replicated across partitions. The device computes the raw
energies (the 64 MB -> 128 KB matvec reduction, the entire data-parallel
workload); the softmax normalization — which is inherently GLOBAL across
all 8 independent cores — happens in the host merge step, in fp64.

Work split across THREE engine groups, all hidden under the ~20 us q DMA
stream (two HWDGE queues, ~212 GB/s each; a 3rd queue measurably HURTS):
  - 20 "normal" tiles [128 tok-grp, 1024 h] for DVE+ACT:
      D tiles (7):  fused scalar_tensor_tensor on DVE (~1.3 us incl
                    DVE_READ_ACCUMULATOR)
      A tiles (13): TT mult on DVE (2x_1p, ~0.69) + ACT Copy+accum
                    (~1.43 us incl ACTIVATION_READ_ACCUMULATOR)
  - 1536 tokens for the PE as HOST-TRANSPOSED qT blocks [128 h, 256 tok]
    (hc-major): lhsT = uT column [128, 1], 8 accumulating matmuls per
    block into PSUM [1, 256]; DVE/ACT copy each block's raw fp32 energies
    to SBUF mid-stream (DMA cannot read PSUM), one [1, 1536] DMA out.
(tensor_tensor_reduce crashes this HW; STT/tensor_reduce are DVE-only per
the compiler engine check; gpsimd TT slows concurrent DVE ops 2.6x via
SBUF port contention and gpsimd cannot touch PSUM — all measured/checked,
all rejected.)

Tail discipline: no device exp. The final chain is just
last-tile-STT -> [128, 20] output DMA, everything else lands earlier.

Precision: q streams as fp16 (host-cast), u host-rounded to fp16; energies
accumulate fp32; softmax in fp64 on host. ~3e-5 scale-rel, gate is 2e-2.

Why NO collectives: on this runner the 8 NEFFs enter ~60 us apart, so ANY
cross-core exchange stalls early cores by the skew. Cores are fully
independent.

Token layout, core r (tokens r*4096 ..): normal part = first 2560 tokens,
partition p holds tokens [20p, 20p+20); PE part = tokens 2560..4095.
outp [128, 20] raw energies; outp2 [1, 1536] raw PE energies.
"""

import numpy as np

S = 32768
H = 1024
NCORES = 8
TPC = S // NCORES  # 4096 tokens per core
NT = 18  # normal tiles (tokens per partition in the normal part)
NTOK = 128 * NT  # 2304 normal tokens per core
PE_TOK = TPC - NTOK  # 1792 PE tokens per core
# PE block sizes in tokens, in block-index order (= qt token order):
# the 128-token pair is sync's late T chunk, processed just before the
# final (scalar T2) block
BLK_SIZES = [256, 256, 256, 256, 256, 128, 128, 256]
assert sum(BLK_SIZES) == PE_TOK
NBLK = len(BLK_SIZES)
OC = H // 128  # 8 h-chunks

# chunk schedule, arrival-interleaved; entries: (queue, kind, n)
#   queue: 0=sync 1=scalar; kind 'N': n normal tiles; 'T': one 256-token
#   qT block (2 tile-units). qT blocks sit mid-early so the PE (and the
#   PSUM->SBUF copies) finish before the tail.
# entries: (queue, kind, n): 'N' = n normal tiles, 'T' = n PE blocks
# (sizes consumed from BLK_SIZES in order). Queues: 0 = sync (fine-grained
# head and tail), 1 = scalar (5 chunks, big lumps mid-stream, ~3.5 us of
# ACT trigger time early). A 3rd (gpsimd) queue measurably splits DMA
# bandwidth evenly across ACTIVE queues and starves the critical one, and
# the gpsimd queue tops out ~110 GB/s — rejected. List order = tile/block
# index order ~= arrival order.
CHUNKS = [
    (0, "N", 1), (0, "T", 1), (0, "T", 1), (1, "T", 2), (0, "N", 2),
    (1, "N", 4), (0, "T", 1), (0, "N", 2), (1, "N", 4), (0, "T", 2),
    (0, "N", 2), (0, "N", 1), (1, "T", 1), (1, "N", 2),
]
assert sum(n for q, k, n in CHUNKS if k == "N") == NT
assert sum(n for q, k, n in CHUNKS if k == "T") == NBLK

# normal-tile engine assignment (tile index = arrival order):
# 7 D (fused DVE), 13 A (DVE mult + ACT reduce); last tile D (fused tail).
ASSIGN = ["A"] * NT
for i in (2, 5, 8, 10, 12, 14, 17):
    ASSIGN[i] = "D"
assert ASSIGN.count("D") == 7

# PSUM->SBUF copies per PE block (0..5): early blocks on ACT (slack
# early), late blocks on DVE. Each copy is emitted into its engine's
# queue after the normal tile index below (so the in-order engine queue
# never stalls on a not-yet-finished PE block).
ACT_COPY_AFTER = {5: 0, 7: 1, 10: 2, 11: 3}
DVE_COPY_AFTER = {9: 4, 12: 5, 13: 6, 15: 7}

_cached = {}


def _build():
    from contextlib import ExitStack

    import concourse.bass as bass
    import concourse.mybir as mybir
    import concourse.tile as tile
    from concourse import bacc

    f32 = mybir.dt.float32
    f16 = mybir.dt.float16
    OP = mybir.AluOpType
    ds = bass.ds

    nc = bacc.Bacc(
        "TRN2", target_bir_lowering=False, debug=False, num_devices=NCORES
    )

    q = nc.dram_tensor("q", [NTOK, H], f16, kind="ExternalInput")
    qt = nc.dram_tensor("qt", [128, OC * PE_TOK], f16, kind="ExternalInput")
    # ur = [u replicated [128, H] | uT [128, OC]]
    ur = nc.dram_tensor("ur", [128, H + OC], f16, kind="ExternalInput")
    outp = nc.dram_tensor("outp", [128, NT], f32, kind="ExternalOutput")
    outp2 = nc.dram_tensor("outp2", [1, PE_TOK], f32, kind="ExternalOutput")

    with tile.TileContext(nc) as tc, ExitStack() as ctx:
        const = ctx.enter_context(tc.tile_pool(name="const", bufs=1))
        qpool = ctx.enter_context(tc.tile_pool(name="qpool", bufs=1))
        work = ctx.enter_context(tc.tile_pool(name="work", bufs=1))
        scr = ctx.enter_context(tc.tile_pool(name="scr", bufs=2))
        psum = ctx.enter_context(tc.tile_pool(name="psum", bufs=1, space="PSUM"))

        # ur leads the sync queue (tiny: 258 KB)
        ur_sb = const.tile([128, H + OC], f16)
        nc.sync.dma_start(ur_sb[:], ur[:])

        q_view = q[:].rearrange("(p t) h -> p (t h)", p=128)
        n_seen = 0
        t_seen = 0
        norm_chunks = []  # (sbuf tile, first tile idx, ntiles)
        qt_chunks = []  # (sbuf tile, block idx)
        for qi, kind, n in CHUNKS:
            eng = nc.sync if qi == 0 else nc.scalar
            if kind == "N":
                cw = n * H
                t_ = qpool.tile([128, cw], f16, tag=f"qn{n_seen}", bufs=1)
                eng.dma_start(t_[:], q_view[:, ds(n_seen * H, cw)])
                norm_chunks.append((t_, n_seen, n))
                n_seen += n
            else:
                base = sum(BLK_SIZES[:t_seen])
                szs = BLK_SIZES[t_seen : t_seen + n]
                cw = OC * sum(szs)
                t_ = qpool.tile([128, cw], f16, tag=f"qt{t_seen}", bufs=1)
                eng.dma_start(t_[:], qt[:, ds(base * OC, cw)])
                off = 0
                for j, sz in enumerate(szs):
                    qt_chunks.append((t_, off, base, t_seen + j, sz))
                    off += OC * sz
                    base += sz
                t_seen += n

        # --- PE path: per block, 8 accumulating rank-1 matmuls
        pe_ps = [None] * NBLK
        pe_base = [0] * NBLK
        for t_, off, base, blk, sz in qt_chunks:
            pb = psum.tile([1, sz], f32, tag=f"pe{blk}", bufs=1)
            for hc in range(OC):
                nc.tensor.matmul(
                    pb[:],
                    lhsT=ur_sb[:, ds(H + hc, 1)],
                    rhs=t_[:, ds(off + hc * sz, sz)],
                    start=(hc == 0),
                    stop=(hc == OC - 1),
                )
            pe_ps[blk] = pb
            pe_base[blk] = base
        pe_sb = work.tile([1, PE_TOK], f32)

        # --- normal tiles: raw energies into e_loc columns
        e_loc = work.tile([128, NT], f32)
        for t_, tbase, ntile in norm_chunks:
            for s_ in range(ntile):
                t_idx = tbase + s_
                src = t_[:, ds(s_ * H, H)]
                if ASSIGN[t_idx] == "A":
                    prod = scr.tile([128, H], f16, tag="proda", bufs=8)
                    nc.vector.tensor_tensor(prod[:], src, ur_sb[:, ds(0, H)],
                                            op=OP.mult)
                    junk = scr.tile([128, H], f16, tag="junk", bufs=2)
                    nc.scalar.activation(
                        junk[:], prod[:], mybir.ActivationFunctionType.Copy,
                        accum_out=e_loc[:, ds(t_idx, 1)],
                    )
                else:
                    prod = scr.tile([128, H], f16, tag="prodd", bufs=4)
                    nc.vector.scalar_tensor_tensor(
                        out=prod[:], in0=src, scalar=1.0,
                        in1=ur_sb[:, ds(0, H)],
                        op0=OP.mult, op1=OP.mult,
                        accum_out=e_loc[:, ds(t_idx, 1)],
                    )
                if t_idx in ACT_COPY_AFTER:
                    blk = ACT_COPY_AFTER[t_idx]
                    nc.scalar.activation(
                        pe_sb[:, ds(pe_base[blk], BLK_SIZES[blk])],
                        pe_ps[blk][:],
                        mybir.ActivationFunctionType.Copy,
                    )
                if t_idx in DVE_COPY_AFTER:
                    blk = DVE_COPY_AFTER[t_idx]
                    nc.vector.tensor_scalar_add(
                        pe_sb[:, ds(pe_base[blk], BLK_SIZES[blk])],
                        pe_ps[blk][:], 0.0
                    )

        nc.sync.dma_start(outp[:], e_loc[:])
        nc.sync.dma_start(outp2[:], pe_sb[:])

    nc.compile()
    return nc


def _get_nc():
    if "nc" not in _cached:
        _cached["nc"] = _build()
    return _cached["nc"]


def make_in_maps(question, W, v):
    qn = np.asarray(question)
    Wn = np.ascontiguousarray(np.asarray(W, dtype=np.float32))
    vn = np.ascontiguousarray(np.asarray(v, dtype=np.float32))
    q16 = np.ascontiguousarray(qn.astype(np.float16))
    u16 = (Wn.T @ vn).astype(np.float16)
    urm = np.empty((128, H + OC), dtype=np.float16)
    urm[:, :H] = u16[None, :]
    urm[:, H:] = u16.reshape(OC, 128).T  # uT[hp, hc] = u[hc*128 + hp]
    in_maps = []
    for r in range(NCORES):
        q_r = q16[r * TPC : (r + 1) * TPC]
        # PE part: [128 hp, block-major, hc-major within block]
        parts = []
        base = NTOK
        for sz in BLK_SIZES:
            blk = q_r[base : base + sz]  # [sz, 1024]
            parts.append(
                blk.reshape(sz, OC, 128).transpose(2, 1, 0).reshape(128, -1)
            )
            base += sz
        qt_r = np.ascontiguousarray(np.concatenate(parts, axis=1))
        in_maps.append({"q": np.ascontiguousarray(q_r[:NTOK]), "qt": qt_r,
                        "ur": urm})
    return in_maps


def run(question, W, v, **spmd_kwargs):
    """Run the SPMD kernel; returns (out [S] fp32, BassKernelResults)."""
    from concourse.bass_utils import run_bass_kernel_spmd

    nc = _get_nc()
    in_maps = make_in_maps(question, W, v)
    res = run_bass_kernel_spmd(nc, in_maps, core_ids=list(range(NCORES)), **spmd_kwargs)
    e = np.empty((NCORES, TPC), dtype=np.float64)
    for r in range(NCORES):
        e[r, :NTOK] = (
            np.asarray(res.results[r]["outp"], dtype=np.float64).reshape(NTOK)
        )
        e[r, NTOK:] = np.asarray(
            res.results[r]["outp2"], dtype=np.float64
        ).reshape(PE_TOK)
    ex = np.exp(e - e.max())
    out = (ex / ex.sum()).reshape(S)
    return out.astype(np.float32), res


def kernel(question, W, b, v):
    out, _ = run(question, W, v)
    return out.reshape(1, 1, S)


# revision 20
# speedup vs baseline: 1.4480x; 1.0276x over previous
"""Trainium2 Bass kernel for nn_Attn: out = softmax_s(v . (W @ q_s + b)).

Algebraic identity:
    energies[s] = v . (W @ q[s] + b) = q[s] . (W^T v) + (v . b)
The (v . b) term is constant and softmax is shift-invariant, so it drops out.
u = W^T v is tiny (H=1024 values, 1/32 of the input bytes / FLOPs); it is
computed on the host in fp32 (alongside the existing host fp16 cast of q)
and shipped replicated across partitions. The device computes the raw
energies (the 64 MB -> 128 KB matvec reduction, the entire data-parallel
workload); the softmax normalization — which is inherently GLOBAL across
all 8 independent cores — happens in the host merge step, in fp64.

Work split across THREE engine groups, all hidden under the ~20 us q DMA
stream (two HWDGE queues, ~212 GB/s each; a 3rd queue measurably HURTS):
  - 20 "normal" tiles [128 tok-grp, 1024 h] for DVE+ACT:
      D tiles (7):  fused scalar_tensor_tensor on DVE (~1.3 us incl
                    DVE_READ_ACCUMULATOR)
      A tiles (13): TT mult on DVE (2x_1p, ~0.69) + ACT Copy+accum
                    (~1.43 us incl ACTIVATION_READ_ACCUMULATOR)
  - 1536 tokens for the PE as HOST-TRANSPOSED qT blocks [128 h, 256 tok]
    (hc-major): lhsT = uT column [128, 1], 8 accumulating matmuls per
    block into PSUM [1, 256]; DVE/ACT copy each block's raw fp32 energies
    to SBUF mid-stream (DMA cannot read PSUM), one [1, 1536] DMA out.
(tensor_tensor_reduce crashes this HW; STT/tensor_reduce are DVE-only per
the compiler engine check; gpsimd TT slows concurrent DVE ops 2.6x via
SBUF port contention and gpsimd cannot touch PSUM — all measured/checked,
all rejected.)

Tail discipline: no device exp. The final chain is just
last-tile-STT -> [128, 20] output DMA, everything else lands earlier.

Precision: q streams as fp16 (host-cast), u host-rounded to fp16; energies
accumulate fp32; softmax in fp64 on host. ~3e-5 scale-rel, gate is 2e-2.

Why NO collectives: on this runner the 8 NEFFs enter ~60 us apart, so ANY
cross-core exchange stalls early cores by the skew. Cores are fully
independent.

Token layout, core r (tokens r*4096 ..): normal part = first 2560 tokens,
partition p holds tokens [20p, 20p+20); PE part = tokens 2560..4095.
outp [128, 20] raw energies; outp2 [1, 1536] raw PE energies.
"""

import numpy as np

S = 32768
H = 1024
NCORES = 8
TPC = S // NCORES  # 4096 tokens per core
NT = 18  # normal tiles (tokens per partition in the normal part)
NTOK = 128 * NT  # 2304 normal tokens per core
PE_TOK = TPC - NTOK  # 1792 PE tokens per core
# PE block sizes in tokens, in block-index order (= qt token order):
# the 128-token pair is sync's late T chunk, processed just before the
# final (scalar T2) block
BLK_SIZES = [256, 256, 256, 256, 256, 256, 128, 128]
assert sum(BLK_SIZES) == PE_TOK
NBLK = len(BLK_SIZES)
OC = H // 128  # 8 h-chunks

# chunk schedule, arrival-interleaved; entries: (queue, kind, n)
#   queue: 0=sync 1=scalar; kind 'N': n normal tiles; 'T': one 256-token
#   qT block (2 tile-units). qT blocks sit mid-early so the PE (and the
#   PSUM->SBUF copies) finish before the tail.
# entries: (queue, kind, n): 'N' = n normal tiles, 'T' = n PE blocks
# (sizes consumed from BLK_SIZES in order). Queues: 0 = sync (fine-grained
# head and tail), 1 = scalar (5 chunks, big lumps mid-stream, ~3.5 us of
# ACT trigger time early). A 3rd (gpsimd) queue measurably splits DMA
# bandwidth evenly across ACTIVE queues and starves the critical one, and
# the gpsimd queue tops out ~110 GB/s — rejected. List order = tile/block
# index order ~= arrival order.
CHUNKS = [
    (0, "N", 1), (1, "N", 1), (0, "N", 2), (1, "N", 2), (0, "T", 1),
    (1, "T", 2), (0, "T", 1), (1, "N", 4), (0, "N", 2), (1, "N", 2),
    (0, "T", 1), (0, "N", 2), (1, "T", 1), (0, "T", 2), (1, "N", 1),
    (0, "N", 1),
]
assert sum(n for q, k, n in CHUNKS if k == "N") == NT
assert sum(n for q, k, n in CHUNKS if k == "T") == NBLK

# normal-tile engine assignment (tile index = arrival order):
# 7 D (fused DVE), 13 A (DVE mult + ACT reduce); last tile D (fused tail).
ASSIGN = ["A"] * NT
for i in (1, 3, 5, 7, 9, 11, 13, 15, 17):
    ASSIGN[i] = "D"
assert ASSIGN.count("D") == 9

# PSUM->SBUF copies per PE block (0..5): early blocks on ACT (slack
# early), late blocks on DVE. Each copy is emitted into its engine's
# queue after the normal tile index below (so the in-order engine queue
# never stalls on a not-yet-finished PE block).
ACT_COPY_AFTER = {6: 0, 8: 1, 10: 2, 11: 3}
DVE_COPY_AFTER = {13: 4, 15: 5, 16: 6, 17: 7}

_cached = {}


def _build():
    from contextlib import ExitStack

    import concourse.bass as bass
    import concourse.mybir as mybir
    import concourse.tile as tile
    from concourse import bacc

    f32 = mybir.dt.float32
    f16 = mybir.dt.float16
    OP = mybir.AluOpType
    ds = bass.ds

    nc = bacc.Bacc(
        "TRN2", target_bir_lowering=False, debug=False, num_devices=NCORES
    )

    q = nc.dram_tensor("q", [NTOK, H], f16, kind="ExternalInput")
    qt = nc.dram_tensor("qt", [128, OC * PE_TOK], f16, kind="ExternalInput")
    # ur = [u replicated [128, H] | uT [128, OC]]
    ur = nc.dram_tensor("ur", [128, H + OC], f16, kind="ExternalInput")
    outp = nc.dram_tensor("outp", [128, NT], f32, kind="ExternalOutput")
    outp2 = nc.dram_tensor("outp2", [1, PE_TOK], f32, kind="ExternalOutput")

    with tile.TileContext(nc) as tc, ExitStack() as ctx:
        const = ctx.enter_context(tc.tile_pool(name="const", bufs=1))
        qpool = ctx.enter_context(tc.tile_pool(name="qpool", bufs=1))
        work = ctx.enter_context(tc.tile_pool(name="work", bufs=1))
        scr = ctx.enter_context(tc.tile_pool(name="scr", bufs=2))
        psum = ctx.enter_context(tc.tile_pool(name="psum", bufs=1, space="PSUM"))

        # ur leads the sync queue (tiny: 258 KB)
        ur_sb = const.tile([128, H + OC], f16)
        nc.sync.dma_start(ur_sb[:], ur[:])

        q_view = q[:].rearrange("(p t) h -> p (t h)", p=128)
        n_seen = 0
        t_seen = 0
        norm_chunks = []  # (sbuf tile, first tile idx, ntiles)
        qt_chunks = []  # (sbuf tile, block idx)
        for qi, kind, n in CHUNKS:
            eng = nc.sync if qi == 0 else nc.scalar
            if kind == "N":
                cw = n * H
                t_ = qpool.tile([128, cw], f16, tag=f"qn{n_seen}", bufs=1)
                eng.dma_start(t_[:], q_view[:, ds(n_seen * H, cw)])
                norm_chunks.append((t_, n_seen, n))
                n_seen += n
            else:
                base = sum(BLK_SIZES[:t_seen])
                szs = BLK_SIZES[t_seen : t_seen + n]
                cw = OC * sum(szs)
                t_ = qpool.tile([128, cw], f16, tag=f"qt{t_seen}", bufs=1)
                eng.dma_start(t_[:], qt[:, ds(base * OC, cw)])
                off = 0
                for j, sz in enumerate(szs):
                    qt_chunks.append((t_, off, base, t_seen + j, sz))
                    off += OC * sz
                    base += sz
                t_seen += n

        # --- PE path: per block, 8 accumulating rank-1 matmuls
        pe_ps = [None] * NBLK
        pe_base = [0] * NBLK
        for t_, off, base, blk, sz in qt_chunks:
            pb = psum.tile([1, sz], f32, tag=f"pe{blk}", bufs=1)
            for hc in range(OC):
                nc.tensor.matmul(
                    pb[:],
                    lhsT=ur_sb[:, ds(H + hc, 1)],
                    rhs=t_[:, ds(off + hc * sz, sz)],
                    start=(hc == 0),
                    stop=(hc == OC - 1),
                )
            pe_ps[blk] = pb
            pe_base[blk] = base
        pe_sb = work.tile([1, PE_TOK], f32)

        # --- normal tiles: raw energies into e_loc columns
        e_loc = work.tile([128, NT], f32)
        for t_, tbase, ntile in norm_chunks:
            for s_ in range(ntile):
                t_idx = tbase + s_
                src = t_[:, ds(s_ * H, H)]
                if ASSIGN[t_idx] == "A":
                    prod = scr.tile([128, H], f16, tag="proda", bufs=8)
                    nc.vector.tensor_tensor(prod[:], src, ur_sb[:, ds(0, H)],
                                            op=OP.mult)
                    junk = scr.tile([128, H], f16, tag="junk", bufs=2)
                    nc.scalar.activation(
                        junk[:], prod[:], mybir.ActivationFunctionType.Copy,
                        accum_out=e_loc[:, ds(t_idx, 1)],
                    )
                else:
                    prod = scr.tile([128, H], f16, tag="prodd", bufs=4)
                    nc.vector.scalar_tensor_tensor(
                        out=prod[:], in0=src, scalar=1.0,
                        in1=ur_sb[:, ds(0, H)],
                        op0=OP.mult, op1=OP.mult,
                        accum_out=e_loc[:, ds(t_idx, 1)],
                    )
                if t_idx in ACT_COPY_AFTER:
                    blk = ACT_COPY_AFTER[t_idx]
                    nc.scalar.activation(
                        pe_sb[:, ds(pe_base[blk], BLK_SIZES[blk])],
                        pe_ps[blk][:],
                        mybir.ActivationFunctionType.Copy,
                    )
                if t_idx in DVE_COPY_AFTER:
                    blk = DVE_COPY_AFTER[t_idx]
                    nc.vector.tensor_scalar_add(
                        pe_sb[:, ds(pe_base[blk], BLK_SIZES[blk])],
                        pe_ps[blk][:], 0.0
                    )

        nc.sync.dma_start(outp[:], e_loc[:])
        nc.sync.dma_start(outp2[:], pe_sb[:])

    nc.compile()
    return nc


def _get_nc():
    if "nc" not in _cached:
        _cached["nc"] = _build()
    return _cached["nc"]


def make_in_maps(question, W, v):
    qn = np.asarray(question)
    Wn = np.ascontiguousarray(np.asarray(W, dtype=np.float32))
    vn = np.ascontiguousarray(np.asarray(v, dtype=np.float32))
    q16 = np.ascontiguousarray(qn.astype(np.float16))
    u16 = (Wn.T @ vn).astype(np.float16)
    urm = np.empty((128, H + OC), dtype=np.float16)
    urm[:, :H] = u16[None, :]
    urm[:, H:] = u16.reshape(OC, 128).T  # uT[hp, hc] = u[hc*128 + hp]
    in_maps = []
    for r in range(NCORES):
        q_r = q16[r * TPC : (r + 1) * TPC]
        # PE part: [128 hp, block-major, hc-major within block]
        parts = []
        base = NTOK
        for sz in BLK_SIZES:
            blk = q_r[base : base + sz]  # [sz, 1024]
            parts.append(
                blk.reshape(sz, OC, 128).transpose(2, 1, 0).reshape(128, -1)
            )
            base += sz
        qt_r = np.ascontiguousarray(np.concatenate(parts, axis=1))
        in_maps.append({"q": np.ascontiguousarray(q_r[:NTOK]), "qt": qt_r,
                        "ur": urm})
    return in_maps


def run(question, W, v, **spmd_kwargs):
    """Run the SPMD kernel; returns (out [S] fp32, BassKernelResults)."""
    from concourse.bass_utils import run_bass_kernel_spmd

    nc = _get_nc()
    in_maps = make_in_maps(question, W, v)
    res = run_bass_kernel_spmd(nc, in_maps, core_ids=list(range(NCORES)), **spmd_kwargs)
    e = np.empty((NCORES, TPC), dtype=np.float64)
    for r in range(NCORES):
        e[r, :NTOK] = (
            np.asarray(res.results[r]["outp"], dtype=np.float64).reshape(NTOK)
        )
        e[r, NTOK:] = np.asarray(
            res.results[r]["outp2"], dtype=np.float64
        ).reshape(PE_TOK)
    ex = np.exp(e - e.max())
    out = (ex / ex.sum()).reshape(S)
    return out.astype(np.float32), res


def kernel(question, W, b, v):
    out, _ = run(question, W, v)
    return out.reshape(1, 1, S)
